# revision 54
# baseline (speedup 1.0000x reference)
"""Self-attention kernel for TRN2, data-parallel over batch (8 cores).

Per core (one batch element), fp8e4 DoubleRow matmuls throughout:
  x loaded fp32 (residual) -> casts to bf16 spread over DVE/GpSimd/ScalarE
  -> TensorE transpose -> DVE copy to xT fp8.
  q/k/v projections fp8 DoubleRow (contraction 2x128 per pass); q/k bias
  via ScalarE Identity epilogue (per-partition), v bias via ones-row
  matmul, v copy via ScalarE.
  Scores computed TRANSPOSED (sT[s,t]) with u-pairs in one DR matmul;
  exp on ScalarE over 2-bank psum groups (scale 1/sqrt(U), shift -2 for
  fp8 range), p stored fp8.
  PV with v as stationary (s-pairs) emits aT[u,t] directly -- no
  attention-matrix transposes.  Row sums via all-ones DR matmul
  replicated across partitions; 1/D (fast reciprocal) folded into the
  aT psum->sbuf copy.  Output projection is a single DR matmul; the
  residual x+ba is pre-combined lazily on the (idle) gpsimd engine so
  the epilogue is one DVE add in fp32.  Block boundaries overlap: the
  next block's scores are emitted before the previous block's PV tail
  and drain so the ScalarE exp stream never starves.
"""

import numpy as np

import concourse.bass as bass
import concourse.mybir as mybir
import concourse.tile as tile
from concourse import bacc
from concourse.bass import ds, ts
from concourse.bass_utils import run_bass_kernel_spmd
from concourse.masks import make_identity

F32 = mybir.dt.float32
BF16 = mybir.dt.bfloat16
F8 = mybir.dt.float8e4
AF = mybir.ActivationFunctionType
DR = mybir.MatmulPerfMode.DoubleRow

B, T, C, U, P = 8, 2048, 512, 256, 128
TC = T // P   # 16 row tiles
CCH = C // P  # 4 c-chunks
UCH = 2       # u-chunks
TBLK = 512    # t-block for attention
NTB = T // TBLK
SCALE = 1.0 / float(np.sqrt(U))
SHIFT = -2.0  # exp(x*SCALE + SHIFT): keeps p in fp8e4 range
# Schraudolph fast-exp on DVE: bits = round(score*FE_M + FE_B) viewed as
# fp32 approximates exp(score*SCALE + SHIFT) to ~2% rms (p is stored in
# fp8e4 with ~3% quantization anyway, and numerator/denominator both use
# the stored values, so softmax stays consistent).
FE_A = 12102203.161561485  # 2^23 / ln 2
FE_M = FE_A * SCALE
FE_B = 1064992212.0 + FE_A * SHIFT

USE_DR = True

_cache = {}


def _dr_matmul(nc, out, lhsT3, rhs3, start, stop):
    """One fp8 DoubleRow matmul over [K,2,M]x[K,2,N], or two plain matmuls."""
    if USE_DR:
        nc.tensor.matmul(out, lhsT=lhsT3, rhs=rhs3, start=start, stop=stop,
                         perf_mode=DR)
    else:
        nc.tensor.matmul(out, lhsT=lhsT3[:, 0], rhs=rhs3[:, 0],
                         start=start, stop=False)
        nc.tensor.matmul(out, lhsT=lhsT3[:, 1], rhs=rhs3[:, 1],
                         start=False, stop=stop)


def _build_kernel(tc):
    nc = tc.nc
    x = nc.dram_tensor("x", [T, C], F32, kind="ExternalInput").ap()
    Wq = nc.dram_tensor("Wq", [C, U], F32, kind="ExternalInput").ap()
    bq = nc.dram_tensor("bq", [U], F32, kind="ExternalInput").ap()
    Wk = nc.dram_tensor("Wk", [C, U], F32, kind="ExternalInput").ap()
    bk = nc.dram_tensor("bk", [U], F32, kind="ExternalInput").ap()
    Wv = nc.dram_tensor("Wv", [C, U], F32, kind="ExternalInput").ap()
    bv = nc.dram_tensor("bv", [U], F32, kind="ExternalInput").ap()
    Wa = nc.dram_tensor("Wa", [U, C], F32, kind="ExternalInput").ap()
    ba = nc.dram_tensor("ba", [C], F32, kind="ExternalInput").ap()
    out = nc.dram_tensor("out", [T, C], F32, kind="ExternalOutput").ap()

    consts = tc.alloc_tile_pool(name="consts", bufs=1)
    persist = tc.alloc_tile_pool(name="persist", bufs=1)

    # --- constants ---
    identity = consts.tile([P, P], BF16)
    make_identity(nc, identity)
    ones_row = consts.tile([1, P], F8)
    nc.vector.memset(ones_row, 1.0)
    ones_pair = consts.tile([P, 2, P], F8)
    nc.vector.memset(ones_pair, 1.0)
    shift_col = consts.tile([P, 1], F32)
    nc.vector.memset(shift_col, SHIFT)

    # --- persistent tensors ---
    x32_sb = persist.tile([P, TC, C], F32)    # x as loaded (fp32)
    x_res = persist.tile([P, TC, C], F32)     # x + ba (residual)
    x_sb = persist.tile([P, TC, C], BF16)     # transpose source
    xT_sb = persist.tile([P, CCH, T], F8)     # x^T  (c on partitions)
    qT_sb = persist.tile([P, UCH, T], F8)     # q^T  (u on partitions)
    kT_sb = persist.tile([P, UCH, T], F8)
    v_sb = persist.tile([P, TC, U], F8)       # v row-major (s, u)
    aT_sb = persist.tile([P, UCH, T], F8)     # a^T normalized
    p_sb = [persist.tile([P, TC, TBLK], F8, name=f"p{i}") for i in range(NTB)]

    # x loads (fp32, HW DGE, alternate queues)
    for tt in range(TC):
        eng = nc.sync if tt % 2 == 0 else nc.scalar
        eng.dma_start(out=x32_sb[:, tt, :], in_=x[ts(tt, P), :])

    # --- weights: fp32 DMA (HW DGE, sync queue) + DVE cast -> fp8 ---
    # layouts: W* [c_lo, cc, u] so cc-pairs (0,1),(2,3) give c/c+128 pairs;
    # Wa [u_lo, uc, c] matching aT's (u_lo, uc) partition layout.
    Wq_s = consts.tile([P, CCH, U], F8)
    Wk_s = consts.tile([P, CCH, U], F8)
    Wv_s = consts.tile([P, CCH, U], F8)
    Wa_s = consts.tile([P, UCH, C], F8)
    bq_sb = consts.tile([P, UCH], F32)
    nc.sync.dma_start(out=bq_sb, in_=bq.rearrange("(uc p) -> p uc", p=P))
    bk_sb = consts.tile([P, UCH], F32)
    nc.sync.dma_start(out=bk_sb, in_=bk.rearrange("(uc p) -> p uc", p=P))
    bv_row = consts.tile([1, U], F8)
    ba_f32 = consts.tile([P, C], F32)   # ba on partition 0
    ba_bc = consts.tile([P, C], F32)    # ba broadcast to all partitions

    with tc.tile_pool(name="wstage", bufs=1) as wstage:
        Wv_f = wstage.tile([P, CCH, U], F32, tag="wv")
        nc.sync.dma_start(out=Wv_f, in_=Wv.rearrange("(cc p) u -> p cc u", p=P))
        nc.vector.tensor_copy(out=Wv_s, in_=Wv_f)
        Wk_f = wstage.tile([P, CCH, U], F32, tag="wk")
        nc.sync.dma_start(out=Wk_f, in_=Wk.rearrange("(cc p) u -> p cc u", p=P))
        nc.vector.tensor_copy(out=Wk_s, in_=Wk_f)
        Wq_f = wstage.tile([P, CCH, U], F32, tag="wq")
        nc.sync.dma_start(out=Wq_f, in_=Wq.rearrange("(cc p) u -> p cc u", p=P))
        nc.vector.tensor_copy(out=Wq_s, in_=Wq_f)
        Wa_f = wstage.tile([P, UCH, C], F32, tag="wa")
        nc.sync.dma_start(out=Wa_f, in_=Wa.rearrange("(uc p) c -> p uc c", p=P))
        nc.vector.tensor_copy(out=Wa_s, in_=Wa_f)
        bv_f = wstage.tile([1, U], F32, tag="bv")
        nc.sync.dma_start(out=bv_f, in_=bv[None, :])
        nc.vector.tensor_copy(out=bv_row, in_=bv_f)
        nc.sync.dma_start(out=ba_f32[0:1, :], in_=ba[None, :])

        # bf16 casts spread over DVE / GpSimd / ScalarE
        for tt in range(TC):
            if tt % 4 == 3:
                nc.scalar.copy(out=x_sb[:, tt, :], in_=x32_sb[:, tt, :])
            elif tt % 4 == 1:
                nc.gpsimd.tensor_copy(out=x_sb[:, tt, :], in_=x32_sb[:, tt, :])
            else:
                nc.vector.tensor_copy(out=x_sb[:, tt, :], in_=x32_sb[:, tt, :])

        # HAM warmup while DMAs land
        with tc.tile_pool(name="warm", bufs=1, space="PSUM") as warm_pool:
            wtile = warm_pool.tile([P, P], F32, name="warmup")
            for i in range(36):
                nc.tensor.matmul(wtile, lhsT=identity, rhs=identity,
                                 start=(i == 0), stop=(i == 35))

        # --- phase 1+2: transpose + projections, per t-block group ---
        with tc.tile_pool(name="tpsum", bufs=2, space="PSUM") as tpsum, \
             tc.tile_pool(name="wpsum", bufs=2, space="PSUM") as wpsum, \
             tc.tile_pool(name="vpsum", bufs=2, space="PSUM") as vpsum:
            for g in range(NTB):
                for half in range(2):
                    tt0 = 4 * g + 2 * half
                    tps = tpsum.tile([P, CCH, 2 * P], BF16, tag="tps")
                    for i in range(2):
                        for cc in range(CCH):
                            nc.tensor.transpose(
                                tps[:, cc, ts(i, P)],
                                x_sb[:, tt0 + i, ts(cc, P)], identity)
                    nc.vector.tensor_copy(out=xT_sb[:, :, ds(tt0 * P, 2 * P)],
                                          in_=tps)
                    for tt in (tt0, tt0 + 1):
                        vps = vpsum.tile([P, U], F32, tag="vps")
                        for cp in range(2):
                            _dr_matmul(nc, vps,
                                       xT_sb[:, ds(2 * cp, 2), ts(tt, P)],
                                       Wv_s[:, ds(2 * cp, 2), :],
                                       start=(cp == 0), stop=False)
                        nc.tensor.matmul(vps, lhsT=ones_row, rhs=bv_row,
                                         start=False, stop=True)
                        nc.scalar.copy(out=v_sb[:, tt, :], in_=vps)
                # q/k projections for this 512-wide t block
                for (W_s, b_sb, dst) in ((Wk_s, bk_sb, kT_sb),
                                         (Wq_s, bq_sb, qT_sb)):
                    for uc in range(UCH):
                        wps = wpsum.tile([P, TBLK], F32, tag="wps")
                        for cp in range(2):
                            _dr_matmul(nc, wps,
                                       W_s[:, ds(2 * cp, 2), ts(uc, P)],
                                       xT_sb[:, ds(2 * cp, 2), ts(g, TBLK)],
                                       start=(cp == 0), stop=(cp == 1))
                        nc.scalar.activation(out=dst[:, uc, ts(g, TBLK)],
                                             in_=wps, func=AF.Identity,
                                             bias=b_sb[:, uc:uc + 1], scale=1.0)

    # --- phase 3: attention ---
    sps_pool = tc.alloc_tile_pool(name="sps", bufs=2, space="PSUM")
    pv_pool = tc.alloc_tile_pool(name="pvps", bufs=2, space="PSUM")
    d_pool = tc.alloc_tile_pool(name="dps", bufs=1, space="PSUM")
    y_psum = tc.alloc_tile_pool(name="ypsum", bufs=1, space="PSUM")
    rcp_pool = tc.alloc_tile_pool(name="rcp", bufs=2)
    y_pool = tc.alloc_tile_pool(name="y", bufs=3)

    # residual with ba pre-added, built lazily on the (idle in attention)
    # gpsimd engine; consumed by the output projections much later.
    nc.gpsimd.partition_broadcast(out_ap=ba_bc, in_ap=ba_f32, channels=P)
    for tt in range(TC):
        nc.gpsimd.tensor_add(out=x_res[:, tt, :], in0=x32_sb[:, tt, :],
                             in1=ba_bc)

    def outproj(tb, tsl, alt_pool=None):
        row0 = tb * TBLK + tsl * P
        if alt_pool is not None:
            yps = alt_pool.tile([P, C], F32, tag="d", name=f"yalt{tsl}")
        else:
            yps = y_psum.tile([P, C], F32, tag="yps")
        _dr_matmul(nc, yps, aT_sb[:, :, ds(row0, P)], Wa_s,
                   start=True, stop=True)
        y_sb = y_pool.tile([P, C], F32, tag="ysb")
        nc.vector.tensor_add(out=y_sb, in0=yps,
                             in1=x_res[:, tb * NTB + tsl, :])
        nc.sync.dma_start(out=out[ds(row0, P), :], in_=y_sb)

    def pv_pair(tb, j, apsT, drep):
        rhs_p = p_sb[tb][:, ds(2 * j, 2), :]
        for uc in range(UCH):
            _dr_matmul(nc, apsT[uc],
                       v_sb[:, ds(2 * j, 2), ts(uc, P)], rhs_p,
                       start=(j == 0), stop=(j == 7))
        _dr_matmul(nc, drep, ones_pair, rhs_p,
                   start=(j == 0), stop=(j == 7))

    fe_pool = tc.alloc_tile_pool(name="fe", bufs=2)

    def scores_exp(tb, j):
        sps_t = sps_pool.tile([P, 2, TBLK], F32, tag="sps",
                              name=f"sps{tb}_{j}")
        for h in range(2):
            _dr_matmul(nc, sps_t[:, h, :],
                       kT_sb[:, :, ts(2 * j + h, P)],
                       qT_sb[:, :, ts(tb, TBLK)],
                       start=True, stop=True)
        if j == 3:
            # fast-exp on DVE to take one pair per block off the ScalarE
            # critical stream
            bits = fe_pool.tile([P, 2 * TBLK], mybir.dt.int32, tag="fe")
            nc.vector.tensor_scalar(out=bits, in0=sps_t, scalar1=FE_M,
                                    scalar2=FE_B, op0=mybir.AluOpType.mult,
                                    op1=mybir.AluOpType.add)
            nc.vector.tensor_copy(out=p_sb[tb][:, ds(2 * j, 2), :],
                                  in_=bits.bitcast(F32))
        else:
            nc.scalar.activation(out=p_sb[tb][:, ds(2 * j, 2), :],
                                 in_=sps_t, func=AF.Exp,
                                 bias=shift_col, scale=SCALE)

    def drain(tb, apsT, drep):
        rcp = rcp_pool.tile([P, TBLK], F32, tag="rcp")
        nc.vector.reciprocal_approx_fast(out=rcp, in_=drep)
        for uc in range(UCH):
            nc.vector.tensor_mul(out=aT_sb[:, uc, ts(tb, TBLK)],
                                 in0=apsT[uc], in1=rcp)

    prev = None
    for tb in range(NTB):
        apsT = [pv_pool.tile([P, TBLK], F32, tag="pv",
                             name=f"apsT{tb}_{uc}") for uc in range(UCH)]
        drep = d_pool.tile([P, TBLK], F32, tag="d", name=f"drep{tb}")
        for j in range(8):
            scores_exp(tb, j)
            if j == 0 and prev is not None:
                # finish the previous block behind this slot's scores so
                # the exp stream never starves at the block boundary
                papsT, pdrep = prev
                pv_pair(tb - 1, 6, papsT, pdrep)
                pv_pair(tb - 1, 7, papsT, pdrep)
                drain(tb - 1, papsT, pdrep)
            if j >= 2:
                pv_pair(tb, j - 2, apsT, drep)
            if tb > 0 and j in (2, 4, 6):
                outproj(tb - 1, j // 2 - 1)
            if tb > 0 and j == 7:
                outproj(tb - 1, 3)
        prev = (apsT, drep)
    pv_pair(NTB - 1, 6, *prev)
    pv_pair(NTB - 1, 7, *prev)
    drain(NTB - 1, *prev)
    for tsl in range(NTB):
        # alternate with the freed D bank so the tail pipelines
        outproj(NTB - 1, tsl, alt_pool=d_pool if tsl % 2 == 1 else None)

    for pool in (fe_pool, y_pool, rcp_pool, y_psum, d_pool, pv_pool,
                 sps_pool, persist, consts):
        pool.release()


def _get_nc():
    if "nc" not in _cache:
        nc = bacc.Bacc("TRN2", target_bir_lowering=False, debug=False)
        with tile.TileContext(nc) as tc:
            _build_kernel(tc)
        nc.compile()
        _cache["nc"] = nc
    return _cache["nc"]


def kernel(**inputs):
    nc = _get_nc()
    shared = {k: np.ascontiguousarray(np.asarray(v, dtype=np.float32))
              for k, v in inputs.items() if k != "x"}
    xs = np.ascontiguousarray(np.asarray(inputs["x"], dtype=np.float32))
    in_maps = [dict(shared, x=xs[b]) for b in range(B)]
    res = run_bass_kernel_spmd(nc, in_maps, core_ids=list(range(B)))
    return np.stack([res.results[b]["out"] for b in range(B)], axis=0)


# revision 55
# speedup vs baseline: 1.0803x; 1.0803x over previous
"""Self-attention kernel for TRN2, data-parallel over batch (8 cores).

Per core (one batch element), fp8e4 DoubleRow matmuls throughout:
  x loaded fp32 (residual) -> casts to bf16 spread over DVE/GpSimd/ScalarE
  -> TensorE transpose -> DVE copy to xT fp8.
  q/k/v projections fp8 DoubleRow (contraction 2x128 per pass); q/k bias
  via ScalarE Identity epilogue (per-partition), v bias via ones-row
  matmul, v copy via ScalarE.
  Scores computed TRANSPOSED (sT[s,t]) with u-pairs in one DR matmul;
  exp on ScalarE over 2-bank psum groups (scale 1/sqrt(U), shift -2 for
  fp8 range), p stored fp8.
  PV with v as stationary (s-pairs) emits aT[u,t] directly -- no
  attention-matrix transposes.  Row sums via all-ones DR matmul
  replicated across partitions; 1/D (fast reciprocal) folded into the
  aT psum->sbuf copy.  Output projection is one DR matmul + ones-row ba
  matmul; epilogue is a single DVE residual add in fp32.
"""

import numpy as np

import concourse.bass as bass
import concourse.mybir as mybir
import concourse.tile as tile
from concourse import bacc
from concourse.bass import ds, ts
from concourse.bass_utils import run_bass_kernel_spmd
from concourse.masks import make_identity

F32 = mybir.dt.float32
BF16 = mybir.dt.bfloat16
F8 = mybir.dt.float8e4
AF = mybir.ActivationFunctionType
DR = mybir.MatmulPerfMode.DoubleRow

B, T, C, U, P = 8, 2048, 512, 256, 128
TC = T // P   # 16 row tiles
CCH = C // P  # 4 c-chunks
UCH = 2       # u-chunks
TBLK = 512    # t-block for attention
NTB = T // TBLK
SCALE = 1.0 / float(np.sqrt(U))
SHIFT = -2.0  # exp(x*SCALE + SHIFT): keeps p in fp8e4 range

USE_DR = True

_cache = {}


def _dr_matmul(nc, out, lhsT3, rhs3, start, stop):
    """One fp8 DoubleRow matmul over [K,2,M]x[K,2,N], or two plain matmuls."""
    if USE_DR:
        nc.tensor.matmul(out, lhsT=lhsT3, rhs=rhs3, start=start, stop=stop,
                         perf_mode=DR)
    else:
        nc.tensor.matmul(out, lhsT=lhsT3[:, 0], rhs=rhs3[:, 0],
                         start=start, stop=False)
        nc.tensor.matmul(out, lhsT=lhsT3[:, 1], rhs=rhs3[:, 1],
                         start=False, stop=stop)


def _build_kernel(tc):
    nc = tc.nc
    x = nc.dram_tensor("x", [T, C], F32, kind="ExternalInput").ap()
    Wq = nc.dram_tensor("Wq", [C, U], F32, kind="ExternalInput").ap()
    bq = nc.dram_tensor("bq", [U], F32, kind="ExternalInput").ap()
    Wk = nc.dram_tensor("Wk", [C, U], F32, kind="ExternalInput").ap()
    bk = nc.dram_tensor("bk", [U], F32, kind="ExternalInput").ap()
    Wv = nc.dram_tensor("Wv", [C, U], F32, kind="ExternalInput").ap()
    bv = nc.dram_tensor("bv", [U], F32, kind="ExternalInput").ap()
    Wa = nc.dram_tensor("Wa", [U, C], F32, kind="ExternalInput").ap()
    ba = nc.dram_tensor("ba", [C], F32, kind="ExternalInput").ap()
    out = nc.dram_tensor("out", [T, C], F32, kind="ExternalOutput").ap()

    consts = tc.alloc_tile_pool(name="consts", bufs=1)
    persist = tc.alloc_tile_pool(name="persist", bufs=1)

    # --- constants ---
    identity = consts.tile([P, P], BF16)
    make_identity(nc, identity)
    ones_row = consts.tile([1, P], F8)
    nc.vector.memset(ones_row, 1.0)
    ones_pair = consts.tile([P, 2, P], F8)
    nc.vector.memset(ones_pair, 1.0)
    shift_col = consts.tile([P, 1], F32)
    nc.vector.memset(shift_col, SHIFT)

    # --- persistent tensors ---
    x32_sb = persist.tile([P, TC, C], F32)    # x as loaded (fp32)
    x_res = persist.tile([P, TC, C], F32)     # x + ba (residual)
    x_sb = persist.tile([P, TC, C], BF16)     # transpose source
    xT_sb = persist.tile([P, CCH, T], F8)     # x^T  (c on partitions)
    qT_sb = persist.tile([P, UCH, T], F8)     # q^T  (u on partitions)
    kT_sb = persist.tile([P, UCH, T], F8)
    v_sb = persist.tile([P, TC, U], F8)       # v row-major (s, u)
    aT_sb = persist.tile([P, UCH, T], F8)     # a^T normalized
    p_sb = [persist.tile([P, TC, TBLK], F8, name=f"p{i}") for i in range(NTB)]

    # x loads (fp32, HW DGE, alternate queues)
    for tt in range(TC):
        eng = nc.sync if tt % 2 == 0 else nc.scalar
        eng.dma_start(out=x32_sb[:, tt, :], in_=x[ts(tt, P), :])

    # --- weights: fp32 DMA (HW DGE, sync queue) + DVE cast -> fp8 ---
    # layouts: W* [c_lo, cc, u] so cc-pairs (0,1),(2,3) give c/c+128 pairs;
    # Wa [u_lo, uc, c] matching aT's (u_lo, uc) partition layout.
    Wq_s = consts.tile([P, CCH, U], F8)
    Wk_s = consts.tile([P, CCH, U], F8)
    Wv_s = consts.tile([P, CCH, U], F8)
    Wa_s = consts.tile([P, UCH, C], F8)
    bq_sb = consts.tile([P, UCH], F32)
    nc.sync.dma_start(out=bq_sb, in_=bq.rearrange("(uc p) -> p uc", p=P))
    bk_sb = consts.tile([P, UCH], F32)
    nc.sync.dma_start(out=bk_sb, in_=bk.rearrange("(uc p) -> p uc", p=P))
    bv_row = consts.tile([1, U], F8)
    ba_f32 = consts.tile([P, C], F32)   # ba on partition 0
    ba_bc = consts.tile([P, C], F32)    # ba broadcast to all partitions

    with tc.tile_pool(name="wstage", bufs=1) as wstage:
        Wv_f = wstage.tile([P, CCH, U], F32, tag="wv")
        nc.sync.dma_start(out=Wv_f, in_=Wv.rearrange("(cc p) u -> p cc u", p=P))
        nc.vector.tensor_copy(out=Wv_s, in_=Wv_f)
        Wk_f = wstage.tile([P, CCH, U], F32, tag="wk")
        nc.sync.dma_start(out=Wk_f, in_=Wk.rearrange("(cc p) u -> p cc u", p=P))
        nc.vector.tensor_copy(out=Wk_s, in_=Wk_f)
        Wq_f = wstage.tile([P, CCH, U], F32, tag="wq")
        nc.sync.dma_start(out=Wq_f, in_=Wq.rearrange("(cc p) u -> p cc u", p=P))
        nc.vector.tensor_copy(out=Wq_s, in_=Wq_f)
        Wa_f = wstage.tile([P, UCH, C], F32, tag="wa")
        nc.sync.dma_start(out=Wa_f, in_=Wa.rearrange("(uc p) c -> p uc c", p=P))
        nc.vector.tensor_copy(out=Wa_s, in_=Wa_f)
        bv_f = wstage.tile([1, U], F32, tag="bv")
        nc.sync.dma_start(out=bv_f, in_=bv[None, :])
        nc.vector.tensor_copy(out=bv_row, in_=bv_f)
        nc.sync.dma_start(out=ba_f32[0:1, :], in_=ba[None, :])

        # bf16 casts spread over DVE / GpSimd / ScalarE
        for tt in range(TC):
            if tt % 4 == 3:
                nc.scalar.copy(out=x_sb[:, tt, :], in_=x32_sb[:, tt, :])
            elif tt % 4 == 1:
                nc.gpsimd.tensor_copy(out=x_sb[:, tt, :], in_=x32_sb[:, tt, :])
            else:
                nc.vector.tensor_copy(out=x_sb[:, tt, :], in_=x32_sb[:, tt, :])

        # HAM warmup while DMAs land
        with tc.tile_pool(name="warm", bufs=1, space="PSUM") as warm_pool:
            wtile = warm_pool.tile([P, P], F32, name="warmup")
            for i in range(36):
                nc.tensor.matmul(wtile, lhsT=identity, rhs=identity,
                                 start=(i == 0), stop=(i == 35))

        # --- phase 1+2: transpose + projections, per t-block group ---
        with tc.tile_pool(name="tpsum", bufs=2, space="PSUM") as tpsum, \
             tc.tile_pool(name="wpsum", bufs=2, space="PSUM") as wpsum, \
             tc.tile_pool(name="vpsum", bufs=2, space="PSUM") as vpsum:
            for g in range(NTB):
                for half in range(2):
                    tt0 = 4 * g + 2 * half
                    tps = tpsum.tile([P, CCH, 2 * P], BF16, tag="tps")
                    for i in range(2):
                        for cc in range(CCH):
                            nc.tensor.transpose(
                                tps[:, cc, ts(i, P)],
                                x_sb[:, tt0 + i, ts(cc, P)], identity)
                    nc.vector.tensor_copy(out=xT_sb[:, :, ds(tt0 * P, 2 * P)],
                                          in_=tps)
                    for tt in (tt0, tt0 + 1):
                        vps = vpsum.tile([P, U], F32, tag="vps")
                        for cp in range(2):
                            _dr_matmul(nc, vps,
                                       xT_sb[:, ds(2 * cp, 2), ts(tt, P)],
                                       Wv_s[:, ds(2 * cp, 2), :],
                                       start=(cp == 0), stop=False)
                        nc.tensor.matmul(vps, lhsT=ones_row, rhs=bv_row,
                                         start=False, stop=True)
                        nc.scalar.copy(out=v_sb[:, tt, :], in_=vps)
                # q/k projections for this 512-wide t block
                for (W_s, b_sb, dst) in ((Wk_s, bk_sb, kT_sb),
                                         (Wq_s, bq_sb, qT_sb)):
                    for uc in range(UCH):
                        wps = wpsum.tile([P, TBLK], F32, tag="wps")
                        for cp in range(2):
                            _dr_matmul(nc, wps,
                                       W_s[:, ds(2 * cp, 2), ts(uc, P)],
                                       xT_sb[:, ds(2 * cp, 2), ts(g, TBLK)],
                                       start=(cp == 0), stop=(cp == 1))
                        nc.scalar.activation(out=dst[:, uc, ts(g, TBLK)],
                                             in_=wps, func=AF.Identity,
                                             bias=b_sb[:, uc:uc + 1], scale=1.0)

    # --- phase 3: attention ---
    sps_pool = tc.alloc_tile_pool(name="sps", bufs=2, space="PSUM")
    pv_pool = tc.alloc_tile_pool(name="pvps", bufs=2, space="PSUM")
    d_pool = tc.alloc_tile_pool(name="dps", bufs=1, space="PSUM")
    y_psum = tc.alloc_tile_pool(name="ypsum", bufs=1, space="PSUM")
    rcp_pool = tc.alloc_tile_pool(name="rcp", bufs=2)
    y_pool = tc.alloc_tile_pool(name="y", bufs=3)

    # residual with ba pre-added, built lazily on the (idle in attention)
    # gpsimd engine; consumed by the output projections much later.
    nc.gpsimd.partition_broadcast(out_ap=ba_bc, in_ap=ba_f32, channels=P)
    for tt in range(TC):
        nc.gpsimd.tensor_add(out=x_res[:, tt, :], in0=x32_sb[:, tt, :],
                             in1=ba_bc)

    def outproj(tb, tsl, alt_pool=None):
        row0 = tb * TBLK + tsl * P
        if alt_pool is not None:
            yps = alt_pool.tile([P, C], F32, tag="d", name=f"yalt{tsl}")
        else:
            yps = y_psum.tile([P, C], F32, tag="yps")
        _dr_matmul(nc, yps, aT_sb[:, :, ds(row0, P)], Wa_s,
                   start=True, stop=True)
        y_sb = y_pool.tile([P, C], F32, tag="ysb")
        nc.vector.tensor_add(out=y_sb, in0=yps,
                             in1=x_res[:, tb * NTB + tsl, :])
        nc.sync.dma_start(out=out[ds(row0, P), :], in_=y_sb)

    def pv_pair(tb, j, apsT, drep):
        rhs_p = p_sb[tb][:, ds(2 * j, 2), :]
        for uc in range(UCH):
            _dr_matmul(nc, apsT[uc],
                       v_sb[:, ds(2 * j, 2), ts(uc, P)], rhs_p,
                       start=(j == 0), stop=(j == 7))
        _dr_matmul(nc, drep, ones_pair, rhs_p,
                   start=(j == 0), stop=(j == 7))

    def scores_exp(tb, j):
        sps_t = sps_pool.tile([P, 2, TBLK], F32, tag="sps",
                              name=f"sps{tb}_{j}")
        for h in range(2):
            _dr_matmul(nc, sps_t[:, h, :],
                       kT_sb[:, :, ts(2 * j + h, P)],
                       qT_sb[:, :, ts(tb, TBLK)],
                       start=True, stop=True)
        nc.scalar.activation(out=p_sb[tb][:, ds(2 * j, 2), :],
                             in_=sps_t, func=AF.Exp,
                             bias=shift_col, scale=SCALE)

    def drain(tb, apsT, drep):
        rcp = rcp_pool.tile([P, TBLK], F32, tag="rcp")
        nc.vector.reciprocal_approx_fast(out=rcp, in_=drep)
        for uc in range(UCH):
            nc.vector.tensor_mul(out=aT_sb[:, uc, ts(tb, TBLK)],
                                 in0=apsT[uc], in1=rcp)

    prev = None
    for tb in range(NTB):
        apsT = [pv_pool.tile([P, TBLK], F32, tag="pv",
                             name=f"apsT{tb}_{uc}") for uc in range(UCH)]
        drep = d_pool.tile([P, TBLK], F32, tag="d", name=f"drep{tb}")
        for j in range(8):
            scores_exp(tb, j)
            if j == 0 and prev is not None:
                # finish the previous block behind this slot's scores so
                # the exp stream never starves at the block boundary
                papsT, pdrep = prev
                pv_pair(tb - 1, 6, papsT, pdrep)
                pv_pair(tb - 1, 7, papsT, pdrep)
                drain(tb - 1, papsT, pdrep)
            if j >= 2:
                pv_pair(tb, j - 2, apsT, drep)
            if tb > 0 and j in (2, 4, 6):
                outproj(tb - 1, j // 2 - 1)
            if tb > 0 and j == 7:
                outproj(tb - 1, 3)
        prev = (apsT, drep)
    pv_pair(NTB - 1, 6, *prev)
    pv_pair(NTB - 1, 7, *prev)
    drain(NTB - 1, *prev)
    for tsl in range(NTB):
        # alternate with the freed D bank so the tail pipelines
        outproj(NTB - 1, tsl, alt_pool=d_pool if tsl % 2 == 1 else None)

    for pool in (y_pool, rcp_pool, y_psum, d_pool, pv_pool, sps_pool,
                 persist, consts):
        pool.release()


def _get_nc():
    if "nc" not in _cache:
        nc = bacc.Bacc("TRN2", target_bir_lowering=False, debug=False)
        with tile.TileContext(nc) as tc:
            _build_kernel(tc)
        nc.compile()
        _cache["nc"] = nc
    return _cache["nc"]


def kernel(**inputs):
    nc = _get_nc()
    shared = {k: np.ascontiguousarray(np.asarray(v, dtype=np.float32))
              for k, v in inputs.items() if k != "x"}
    xs = np.ascontiguousarray(np.asarray(inputs["x"], dtype=np.float32))
    in_maps = [dict(shared, x=xs[b]) for b in range(B)]
    res = run_bass_kernel_spmd(nc, in_maps, core_ids=list(range(B)))
    return np.stack([res.results[b]["out"] for b in range(B)], axis=0)
